# revision 4
# baseline (speedup 1.0000x reference)
"""Multi-head self-attention (N=4, T=2048, D=1024, H=16) on 8 TRN2 NeuronCores.

Sharding: core c -> (batch n = c//2, head-group g = c%2 of 8 heads).
Each core projects its batch with its 512-row slices of Wq/Wk/Wv, runs
attention for its 8 heads, AllGathers the per-pair context (bf16), and
computes its 512 output columns with its slice of Wo.
"""

from contextlib import ExitStack

import numpy as np

import concourse.bass as bass
import concourse.mybir as mybir
import concourse.tile as tile
from concourse import bacc
from concourse.bass_utils import run_bass_kernel_spmd
from concourse.masks import make_identity

N, T, D, H, DH = 4, 2048, 1024, 16, 64
N_CORES = 8
G = 512            # per-core projection width (8 heads x 64)
HPC = 8            # heads per core
SCALE = 1.0 / 8.0  # 1/sqrt(DH)

f32 = mybir.dt.float32
f32r = mybir.dt.float32r
bf16 = mybir.dt.bfloat16
i32 = mybir.dt.int32

COMPUTE_DT = "f32r"  # {"f32r", "bf16"} dtype family for projection/S/out matmuls


def build_nc(compute_dt: str = COMPUTE_DT) -> bacc.Bacc:
    # float32r tiles: PE runs 1 cycle/row (vs 4 for fp32); producers
    # (DVE/ACT evicts) round to f32r precision on write as the BIR verifier
    # requires.
    cdt = f32r if compute_dt == "f32r" else bf16

    def mm(ap):
        return ap

    nc = bacc.Bacc(
        "TRN2", target_bir_lowering=False, debug=False, num_devices=N_CORES
    )
    x_d = nc.dram_tensor("query", [T, D], f32, kind="ExternalInput").ap()
    m_d = nc.dram_tensor("mask", [T], i32, kind="ExternalInput").ap()
    wq_d = nc.dram_tensor("Wq", [G, D], f32, kind="ExternalInput").ap()
    wk_d = nc.dram_tensor("Wk", [G, D], f32, kind="ExternalInput").ap()
    wv_d = nc.dram_tensor("Wv", [G, D], f32, kind="ExternalInput").ap()
    wo_d = nc.dram_tensor("Wo", [G, D], f32, kind="ExternalInput").ap()
    bq_d = nc.dram_tensor("bq", [G], f32, kind="ExternalInput").ap()
    bk_d = nc.dram_tensor("bk", [G], f32, kind="ExternalInput").ap()
    bv_d = nc.dram_tensor("bv", [G, ], f32, kind="ExternalInput").ap()
    bo_d = nc.dram_tensor("bo", [G], f32, kind="ExternalInput").ap()
    out_d = nc.dram_tensor("out", [T, G], f32, kind="ExternalOutput").ap()

    TB = T // 128   # 16 token blocks
    DB = D // 128   # 8 feature blocks
    GB = G // 128   # 4 projected blocks

    with tile.TileContext(nc) as tc, ExitStack() as ctx:
        const = ctx.enter_context(tc.tile_pool(name="const", bufs=1))
        identity = const.tile([128, 128], f32)
        make_identity(nc, identity)
        maskb = const.tile([128, T], f32, tag="maskb")
        bq_c = const.tile([128, GB], f32, tag="bq")
        bk_c = const.tile([128, GB], f32, tag="bk")
        bvb = const.tile([128, G], f32, tag="bvb")
        bob = const.tile([128, G], f32, tag="bob")

        qpool = ctx.enter_context(tc.tile_pool(name="qpool", bufs=1))
        q_t = [qpool.tile([128, T], cdt, tag=f"q{i}", name=f"q{i}") for i in range(GB)]
        k_t = [qpool.tile([128, T], cdt, tag=f"k{i}", name=f"k{i}") for i in range(GB)]
        v_t = [qpool.tile([128, HPC * 65], bf16, tag=f"v{i}", name=f"v{i}") for i in range(TB)]

        dram = ctx.enter_context(tc.tile_pool(name="dram", bufs=1, space="DRAM"))
        cc_in = dram.tile([G, T], bf16)
        cc_out = dram.tile([2 * G, T], bf16)

        # ---- mask + biases ----
        with tc.tile_pool(name="mload", bufs=1) as mp:
            m_i = mp.tile([1, T], i32)
            nc.sync.dma_start(m_i[:], m_d[None, :])
            m_f = mp.tile([1, T], f32)
            nc.vector.tensor_copy(m_f[:], m_i[:])
            nc.gpsimd.partition_broadcast(maskb[:], m_f[:])
            nc.sync.dma_start(bq_c[:], bq_d.rearrange("(j p) -> p j", p=128))
            nc.sync.dma_start(bk_c[:], bk_d.rearrange("(j p) -> p j", p=128))
            bv_r = mp.tile([1, G], f32, tag="bvr")
            nc.sync.dma_start(bv_r[:], bv_d[None, :])
            nc.gpsimd.partition_broadcast(bvb[:], bv_r[:])
            bo_r = mp.tile([1, G], f32, tag="bor")
            nc.sync.dma_start(bo_r[:], bo_d[None, :])
            nc.gpsimd.partition_broadcast(bob[:], bo_r[:])

        # ---- phase 1: X^T, W^T, projections ----
        with tc.tile_pool(name="xt", bufs=1) as xtp, \
             tc.tile_pool(name="stage", bufs=3) as sp, \
             tc.tile_pool(name="wt", bufs=16) as wtp, \
             tc.tile_pool(name="pp", bufs=4, space="PSUM") as pp:
            xt = [xtp.tile([128, T], cdt, tag=f"xt{d}", name=f"xt{d}") for d in range(DB)]
            for i in range(TB):
                xs = sp.tile([128, D], f32, tag="stage")
                nc.sync.dma_start(xs[:], x_d[i * 128:(i + 1) * 128, :])
                for d in range(DB):
                    ps = pp.tile([128, 512], f32, tag="pp")
                    nc.tensor.transpose(
                        ps[:, 0:128], xs[:, d * 128:(d + 1) * 128], identity[:]
                    )
                    nc.any.tensor_copy(
                        xt[d][:, i * 128:(i + 1) * 128], ps[:, 0:128]
                    )

            def load_wT(w_dram):
                tiles = [wtp.tile([128, G], cdt, tag="wt", name="wt") for _ in range(DB)]
                for r in range(GB):
                    ws = sp.tile([128, D], f32, tag="stage")
                    nc.sync.dma_start(ws[:], w_dram[r * 128:(r + 1) * 128, :])
                    for d in range(DB):
                        ps = pp.tile([128, 512], f32, tag="pp")
                        nc.tensor.transpose(
                            ps[:, 0:128], ws[:, d * 128:(d + 1) * 128], identity[:]
                        )
                        nc.any.tensor_copy(
                            tiles[d][:, r * 128:(r + 1) * 128], ps[:, 0:128]
                        )
                return tiles

            # Q^T[g*128+p, t], masked-query columns zeroed so their scores
            # are all equal (-> uniform softmax, matching the -1e20 fill).
            wqT = load_wT(wq_d)
            for b in range(GB):
                for tch in range(4):
                    ps = pp.tile([128, 512], f32, tag="pp")
                    for d in range(DB):
                        nc.tensor.matmul(
                            ps[:],
                            mm(wqT[d][:, b * 128:(b + 1) * 128]),
                            mm(xt[d][:, tch * 512:(tch + 1) * 512]),
                            start=(d == 0),
                            stop=(d == DB - 1),
                        )
                    nc.vector.scalar_tensor_tensor(
                        q_t[b][:, tch * 512:(tch + 1) * 512],
                        ps[:],
                        bq_c[:, b:b + 1],
                        maskb[:, tch * 512:(tch + 1) * 512],
                        op0=mybir.AluOpType.add,
                        op1=mybir.AluOpType.mult,
                    )
            wkT = load_wT(wk_d)
            for b in range(GB):
                for tch in range(4):
                    ps = pp.tile([128, 512], f32, tag="pp")
                    for d in range(DB):
                        nc.tensor.matmul(
                            ps[:],
                            mm(wkT[d][:, b * 128:(b + 1) * 128]),
                            mm(xt[d][:, tch * 512:(tch + 1) * 512]),
                            start=(d == 0),
                            stop=(d == DB - 1),
                        )
                    nc.vector.tensor_scalar_add(
                        k_t[b][:, tch * 512:(tch + 1) * 512], ps[:], bk_c[:, b:b + 1]
                    )
            # V token-major [t, dout] with a ones column appended per head
            wvT = load_wT(wv_d)
            for i in range(TB):
                nc.gpsimd.memset(v_t[i][:], 1.0)
                ps = pp.tile([128, 512], f32, tag="pp")
                for d in range(DB):
                    nc.tensor.matmul(
                        ps[:],
                        mm(xt[d][:, i * 128:(i + 1) * 128]),
                        mm(wvT[d][:]),
                        start=(d == 0),
                        stop=(d == DB - 1),
                    )
                for h in range(HPC):
                    nc.vector.tensor_tensor(
                        v_t[i][:, h * 65:h * 65 + 64],
                        ps[:, h * 64:(h + 1) * 64],
                        bvb[:, h * 64:(h + 1) * 64],
                        op=mybir.AluOpType.add,
                    )

        # ---- phase 2: attention, unit = (head, tq half of 1024) ----
        with tc.tile_pool(name="slab", bufs=2) as slabp, \
             tc.tile_pool(name="zbuf", bufs=2) as zp, \
             tc.tile_pool(name="cstage", bufs=3) as csp, \
             tc.tile_pool(name="spsum", bufs=2, space="PSUM") as spp, \
             tc.tile_pool(name="cpsum", bufs=2, space="PSUM") as cpp:
            for h in range(HPC):
                qk = h // 2
                hb = (h % 2) * 64
                for tqh in range(2):
                    t0 = tqh * 1024
                    slab = slabp.tile([128, 16 * 1024], bf16, tag="slab")
                    for j in range(TB):
                        sps = spp.tile([128, 1024], f32, tag="sp")
                        for q in range(2):
                            nc.tensor.matmul(
                                sps[:, q * 512:(q + 1) * 512],
                                mm(k_t[qk][hb:hb + 64, j * 128:(j + 1) * 128]),
                                mm(q_t[qk][hb:hb + 64, t0 + q * 512:t0 + (q + 1) * 512]),
                                start=True,
                                stop=True,
                            )
                        nc.scalar.activation(
                            slab[:, j * 1024:(j + 1) * 1024],
                            sps[:],
                            mybir.ActivationFunctionType.Exp,
                            scale=SCALE,
                        )
                    cps = cpp.tile([65, 1024], f32, tag="cp")
                    for q in range(2):
                        for j in range(TB):
                            nc.tensor.matmul(
                                cps[:, q * 512:(q + 1) * 512],
                                v_t[j][:, h * 65:h * 65 + 65],
                                slab[:, j * 1024 + q * 512:j * 1024 + (q + 1) * 512],
                                start=(j == 0),
                                stop=(j == TB - 1),
                            )
                    # softmax denominator: row 64 = sum_k exp; normalize ctx
                    zrow = zp.tile([128, 1024], f32, tag="z")
                    nc.vector.tensor_copy(zrow[64:65, :], cps[64:65, :])
                    nc.sync.dma_start(zrow[0:1, :], zrow[64:65, :])
                    inv = zp.tile([1, 1024], f32, tag="zi")
                    nc.vector.reciprocal(inv[:], zrow[0:1, :])
                    bct = zp.tile([64, 1024], f32, tag="bc")
                    nc.gpsimd.partition_broadcast(bct[:], inv[:])
                    cst = csp.tile([64, 1024], bf16, tag="cst", name="cst")
                    nc.vector.tensor_tensor(
                        cst[:],
                        cps[0:64, :],
                        bct[:],
                        op=mybir.AluOpType.mult,
                    )
                    nc.sync.dma_start(
                        cc_in[h * 64:(h + 1) * 64, t0:t0 + 1024], cst[:]
                    )

        # ---- phase 3: pair AllGather + output projection ----
        nc.gpsimd.collective_compute(
            "AllGather",
            mybir.AluOpType.bypass,
            replica_groups=[[0, 1], [2, 3], [4, 5], [6, 7]],
            ins=[cc_in[:].opt()],
            outs=[cc_out[:].opt()],
        )
        with tc.tile_pool(name="cf", bufs=1) as cfp, \
             tc.tile_pool(name="stage3", bufs=3) as sp3, \
             tc.tile_pool(name="wot", bufs=8) as wotp, \
             tc.tile_pool(name="op", bufs=4, space="PSUM") as opp:
            cf = [cfp.tile([128, T], bf16, tag=f"cf{d}", name=f"cf{d}") for d in range(DB)]
            for d in range(DB):
                nc.sync.dma_start(cf[d][:], cc_out[d * 128:(d + 1) * 128, :])
            woT = [wotp.tile([128, G], bf16, tag="wot", name="wot") for _ in range(DB)]
            for r in range(GB):
                ws = sp3.tile([128, D], f32, tag="stage3")
                nc.sync.dma_start(ws[:], wo_d[r * 128:(r + 1) * 128, :])
                for d in range(DB):
                    ps = opp.tile([128, 512], f32, tag="op")
                    nc.tensor.transpose(
                        ps[:, 0:128], ws[:, d * 128:(d + 1) * 128], identity[:]
                    )
                    nc.any.tensor_copy(woT[d][:, r * 128:(r + 1) * 128], ps[:, 0:128])
            for i in range(TB):
                ps = opp.tile([128, 512], f32, tag="op")
                for d in range(DB):
                    nc.tensor.matmul(
                        ps[:],
                        cf[d][:, i * 128:(i + 1) * 128],
                        woT[d][:],
                        start=(d == 0),
                        stop=(d == DB - 1),
                    )
                os_ = sp3.tile([128, G], f32, tag="ostage")
                nc.vector.tensor_tensor(os_[:], ps[:], bob[:], op=mybir.AluOpType.add)
                nc.sync.dma_start(out_d[i * 128:(i + 1) * 128, :], os_[:])

    nc.compile()
    return nc


def shard_inputs(query, mask, Wq, bq, Wk, bk, Wv, bv, Wo, bo):
    in_maps = []
    for c in range(N_CORES):
        n, g = c // 2, c % 2
        sl = slice(g * G, (g + 1) * G)
        in_maps.append(
            {
                "query": np.ascontiguousarray(query[n], dtype=np.float32),
                "mask": np.ascontiguousarray(mask[n], dtype=np.int32),
                "Wq": np.ascontiguousarray(Wq[sl], dtype=np.float32),
                "Wk": np.ascontiguousarray(Wk[sl], dtype=np.float32),
                "Wv": np.ascontiguousarray(Wv[sl], dtype=np.float32),
                "Wo": np.ascontiguousarray(Wo[sl], dtype=np.float32),
                "bq": np.ascontiguousarray(bq[sl], dtype=np.float32),
                "bk": np.ascontiguousarray(bk[sl], dtype=np.float32),
                "bv": np.ascontiguousarray(bv[sl], dtype=np.float32),
                "bo": np.ascontiguousarray(bo[sl], dtype=np.float32),
            }
        )
    return in_maps


def gather_outputs(results):
    out = np.empty((N, T, D), np.float32)
    for c in range(N_CORES):
        n, g = c // 2, c % 2
        out[n][:, g * G:(g + 1) * G] = results[c]["out"]
    return out


def kernel(query, mask, Wq, bq, Wk, bk, Wv, bv, Wo, bo):
    in_maps = shard_inputs(query, mask, Wq, bq, Wk, bk, Wv, bv, Wo, bo)
    nc = build_nc()
    res = run_bass_kernel_spmd(nc, in_maps, list(range(N_CORES)))
    return gather_outputs(res.results)


if __name__ == "__main__":
    rng = np.random.default_rng(0)
    ins = {
        "query": rng.standard_normal((N, T, D), np.float32),
        "mask": rng.integers(0, 2, (N, T)).astype(np.int32),
        "Wq": rng.standard_normal((D, D), np.float32) / 32,
        "bq": np.zeros(D, np.float32),
        "Wk": rng.standard_normal((D, D), np.float32) / 32,
        "bk": np.zeros(D, np.float32),
        "Wv": rng.standard_normal((D, D), np.float32) / 32,
        "bv": np.zeros(D, np.float32),
        "Wo": rng.standard_normal((D, D), np.float32) / 32,
        "bo": np.zeros(D, np.float32),
    }
    out = kernel(**ins)
    print(out.shape, out.dtype)
